# revision 21
# baseline (speedup 1.0000x reference)
"""Trainium2 Bass kernel for EnhancedLinkPredictor (GNN common-neighbor link prediction).

Math (per prediction edge e=(s,d)):
  shared_ddi = adj_ddi[s] & adj_ddi[d]          (drug-drug, N_D=8192)
  cn_ddi     = (shared_ddi @ z_drug)  / max(|shared_ddi|, 1)
  shared_dp  = adj_dp[s]  & adj_dp[d]           (drug-protein, N_P=4096)
  cn_prot    = (shared_dp @ z_protein) / max(|shared_dp|, 1)
  pair  = [z_drug[s], z_drug[d], cn_ddi, cn_prot]   (256)
  out   = sigmoid(relu(pair @ W1 + b1) @ W2 + b2)

Device strategy (8 cores, data-parallel over the 16384 pred edges, 2048/core):
  - Adjacency rows are fp8-coded bytes {0x00, 0x38=1.0fp8}, built host-side and
    compacted per core to the rows that core actually touches (<=4096 unique).
    The ddi row additionally carries the drug's z row as 128 bf16 (256B), so a
    single gather per edge endpoint fetches adjacency + embedding.
  - dma_gather(transpose=True, uint16 view) pulls whole rows per edge endpoint
    and lands them k-pair-per-partition (partition p of 256-k chunk f holds
    k = 256f+2p, 256f+2p+1) - fp8-DoubleRow-ready, no on-chip transpose.
    GPSIMD descriptor generation costs ~8.4ns/idx, so each endpoint is visited
    exactly once per relation (full-row elems, small idx lists per call).
  - AND between s-rows and d-rows is a uint32 bitwise_and on DVE, writing
    128/256-edge column slices of 512-edge masked supertiles.
  - Matmuls run in fp8 DoubleRow with Z split into two e4m3 limbs (hi +
    residual) for ~bf16 accuracy, accumulated in f32 PSUM; Z column 64 is ones
    so the intersection counts fall out of the same matmul.
  - Everything downstream is computed transposed ([dim, edge]) so each
    matmul's contraction dim is on partitions; the MLP consumes pair^T.
"""

import numpy as np
import ml_dtypes
from contextlib import ExitStack

import concourse.bass as bass
import concourse.bacc as bacc
import concourse.mybir as mybir
import concourse.tile as tile

N_D, N_P = 8192, 4096
D_DIM, HID = 64, 128
E_PRED = 16384
N_CORES = 8
E_LOC = E_PRED // N_CORES          # 2048 edges per core
U_PAD = 4096                       # compacted adjacency row count (>= unique refs)

MCOL = 80                          # padded Z columns (64 dims + 1 ones + 15 pad)
KCHUNK = 256                       # k per DoubleRow matmul

F_DDI = N_D // KCHUNK              # 32 adjacency chunks in a ddi row
F_DDIZ = F_DDI + 1                 # + z chunk
F_DP = N_P // KCHUNK               # 16 chunks in a dp row
G_DDI = 128                        # edges per ddi gather call (266 descs)
G_DP = 256                         # edges per dp gather call (258 descs)
N_G = E_LOC // G_DDI               # 16 ddi edge groups
N_H = E_LOC // G_DP                # 8 dp edge groups
N_ET = E_LOC // 512                # 4 supertiles of 512 edges
IDX_COLS = 2 * N_G * (G_DDI // 16) + 2 * N_H * (G_DP // 16)  # 512
USE_DOUBLEROW = True

FP8 = ml_dtypes.float8_e4m3
BF16 = ml_dtypes.bfloat16


def _fp8_limbs(x: np.ndarray):
    hi = x.astype(FP8)
    lo = (x - hi.astype(np.float32)).astype(FP8)
    return hi.view(np.uint8), lo.view(np.uint8)


def _pack_z_doublerow(z: np.ndarray):
    """z [K, 64] f32 -> [2*K/256, 128, 2*MCOL] uint8 fp8-coded DoubleRow lhsT
    groups (limb-major), with a ones column at index 64."""
    K = z.shape[0]
    zext = np.zeros((K, MCOL), dtype=np.float32)
    zext[:, :D_DIM] = z
    zext[:, D_DIM] = 1.0
    hi, lo = _fp8_limbs(zext)
    n_chunks = K // KCHUNK
    out = np.empty((2 * n_chunks, 128, 2 * MCOL), dtype=np.uint8)
    for li, limb in enumerate((hi, lo)):
        out[li * n_chunks:(li + 1) * n_chunks] = limb.reshape(
            n_chunks, 128, 2 * MCOL
        )
    return out


def _wrap_idxs(idx: np.ndarray):
    """[n] int -> [128, n/16] int16 wrapped (j -> [j%16, j//16]) + 8x replicated."""
    n = idx.shape[0]
    w = np.zeros((16, n // 16), dtype=np.int16)
    w[np.arange(n) % 16, np.arange(n) // 16] = idx.astype(np.int16)
    return np.tile(w, (8, 1))


def build_body(tc, t):
    """Emit the per-core program. t: dict name -> AP of DRAM tensors."""
    nc = tc.nc
    dt = mybir.dt
    with ExitStack() as ctx:
        const = ctx.enter_context(tc.tile_pool(name="const", bufs=1))
        gpool = ctx.enter_context(tc.tile_pool(name="gath", bufs=6))
        mpool = ctx.enter_context(tc.tile_pool(name="mask", bufs=2))
        tails = ctx.enter_context(tc.tile_pool(name="tails", bufs=2))
        pairp = ctx.enter_context(tc.tile_pool(name="pair", bufs=1))
        psum = ctx.enter_context(tc.tile_pool(name="ps", bufs=8, space="PSUM"))

        # ---- constants / small loads ----
        idxt = const.tile([128, IDX_COLS], dt.int16)
        nc.sync.dma_start(idxt[:], t["IDX"][:, :])

        zd = const.tile([128, 2 * F_DDI * 2 * MCOL], dt.uint8)
        nc.sync.dma_start(zd[:], t["ZD"][:, :])
        zp = const.tile([128, 2 * F_DP * 2 * MCOL], dt.uint8)
        nc.sync.dma_start(zp[:], t["ZP"][:, :])
        w1t = const.tile([64, 4 * HID], dt.uint16)
        nc.sync.dma_start(w1t[:], t["W1"][:, :])
        w2t = const.tile([128, 1], dt.uint16)
        nc.sync.dma_start(w2t[:], t["W2"][:, :])
        b1t = const.tile([128, 1], dt.float32)
        nc.sync.dma_start(b1t[:], t["B1"][:, :])
        b2t = const.tile([1, 1], dt.float32)
        nc.sync.dma_start(b2t[:], t["B2"][:, :])
        ones = const.tile([1, 64], dt.float32)
        nc.vector.memset(ones[:], 1.0)

        zsrc = pairp.tile([128, E_LOC], dt.uint16)
        zdst = pairp.tile([128, E_LOC], dt.uint16)

        cn_ps = {}
        for et in range(N_ET):
            cn_ps[("ddi", et)] = psum.tile(
                [MCOL, 512], dt.float32, tag="ps", name=f"cn_ddi{et}"
            )
            cn_ps[("dp", et)] = psum.tile(
                [MCOL, 512], dt.float32, tag="ps", name=f"cn_dp{et}"
            )

        def matmuls(rel, et, msk, zt, n_chunks):
            m8 = msk[:].bitcast(dt.float8e4).rearrange(
                "p (f i two) -> p f two i", f=n_chunks, two=2
            )
            zt8 = zt[:].bitcast(dt.float8e4).rearrange(
                "p (g two m) -> p g two m", g=2 * n_chunks, two=2
            )
            for f in range(n_chunks):
                for limb in range(2):
                    g = limb * n_chunks + f
                    first = f == 0 and limb == 0
                    last = f == n_chunks - 1 and limb == 1
                    if USE_DOUBLEROW:
                        nc.tensor.matmul(
                            cn_ps[(rel, et)][:],
                            zt8[:, g, :, :],
                            m8[:, f, :, :],
                            start=first,
                            stop=last,
                            perf_mode=mybir.MatmulPerfMode.DoubleRow,
                        )
                    else:
                        for par in range(2):
                            nc.tensor.matmul(
                                cn_ps[(rel, et)][:],
                                zt8[:, g, par, :],
                                m8[:, f, par, :],
                                start=first and par == 0,
                                stop=last and par == 1,
                            )

        # ---- ddi (+z) stream: 2 gathers per 128-edge group ----
        for et in range(N_ET):
            msk = mpool.tile([128, F_DDI * 512], dt.uint16, tag="mka", name=f"mka{et}")
            for sub in range(4):
                g = 4 * et + sub
                pair_tiles = []
                for side in range(2):  # 0=s, 1=d
                    gt = gpool.tile([128, F_DDIZ * G_DDI], dt.uint16, tag="gt")
                    col0 = (2 * g + side) * (G_DDI // 16)
                    nc.gpsimd.dma_gather(
                        out_ap=gt[:].rearrange("p (f i) -> p f i", f=F_DDIZ),
                        in_ap=t["A1"][:, :],
                        idxs_ap=idxt[:, col0:col0 + G_DDI // 16],
                        num_idxs=G_DDI,
                        num_idxs_reg=G_DDI,
                        elem_size=F_DDIZ * 128,
                        elem_step=F_DDIZ * 128,
                        transpose=True,
                        single_packet=False,
                    )
                    pair_tiles.append(gt)
                    # z chunk -> pair^T rows
                    ztile = zsrc if side == 0 else zdst
                    nc.vector.tensor_copy(
                        ztile[:, G_DDI * g:G_DDI * (g + 1)],
                        gt[:].rearrange("p (f i) -> p f i", f=F_DDIZ)[:, F_DDI, :],
                    )
                s32 = pair_tiles[0][:].bitcast(dt.uint32).rearrange(
                    "p (f i) -> p f i", f=F_DDIZ
                )
                d32 = pair_tiles[1][:].bitcast(dt.uint32).rearrange(
                    "p (f i) -> p f i", f=F_DDIZ
                )
                m32 = msk[:].bitcast(dt.uint32).rearrange("p (f i) -> p f i", f=F_DDI)
                nw = G_DDI // 2
                nc.vector.tensor_tensor(
                    m32[:, :, sub * nw:(sub + 1) * nw],
                    s32[:, :F_DDI, :],
                    d32[:, :F_DDI, :],
                    mybir.AluOpType.bitwise_and,
                )
            matmuls("ddi", et, msk, zd, F_DDI)

        # ---- dp stream: 2 gathers per 256-edge group ----
        dp_base = 2 * N_G * (G_DDI // 16)
        for et in range(N_ET):
            msk = mpool.tile([128, F_DP * 512], dt.uint16, tag="mkb", name=f"mkb{et}")
            for sub in range(2):
                h = 2 * et + sub
                pair_tiles = []
                for side in range(2):
                    gt = gpool.tile([128, F_DP * G_DP], dt.uint16, tag="gt")
                    col0 = dp_base + (2 * h + side) * (G_DP // 16)
                    nc.gpsimd.dma_gather(
                        out_ap=gt[:].rearrange("p (f i) -> p f i", f=F_DP),
                        in_ap=t["A2"][:, :],
                        idxs_ap=idxt[:, col0:col0 + G_DP // 16],
                        num_idxs=G_DP,
                        num_idxs_reg=G_DP,
                        elem_size=F_DP * 128,
                        elem_step=F_DP * 128,
                        transpose=True,
                        single_packet=False,
                    )
                    pair_tiles.append(gt)
                s32 = pair_tiles[0][:].bitcast(dt.uint32).rearrange(
                    "p (f i) -> p f i", f=F_DP
                )
                d32 = pair_tiles[1][:].bitcast(dt.uint32).rearrange(
                    "p (f i) -> p f i", f=F_DP
                )
                m32 = msk[:].bitcast(dt.uint32).rearrange("p (f i) -> p f i", f=F_DP)
                nw = G_DP // 2
                nc.vector.tensor_tensor(
                    m32[:, :, sub * nw:(sub + 1) * nw],
                    s32[:, :, :],
                    d32[:, :, :],
                    mybir.AluOpType.bitwise_and,
                )
            matmuls("dp", et, msk, zp, F_DP)

        # ---- normalize: cn / max(count, 1), into bf16 pair^T tiles ----
        cn_sb = {
            "ddi": pairp.tile([64, E_LOC], dt.bfloat16, tag="cnddi", name="cnddi"),
            "dp": pairp.tile([64, E_LOC], dt.bfloat16, tag="cndp", name="cndp"),
        }
        for et in range(N_ET):
            for rel in ("ddi", "dp"):
                ps = cn_ps[(rel, et)]
                raw = tails.tile([66, 512], dt.float32, tag="raw")
                nc.scalar.copy(raw[:], ps[0:66, :])
                rec = tails.tile([1, 512], dt.float32, tag="rec")
                nc.vector.tensor_scalar_max(rec[:], raw[64:65, :], 1.0)
                nc.vector.reciprocal(rec[:], rec[:])
                bc = psum.tile([64, 512], dt.float32, tag="ps")
                nc.tensor.matmul(bc[:], ones[:], rec[:], start=True, stop=True)
                nc.vector.tensor_tensor(
                    cn_sb[rel][:, 512 * et:512 * (et + 1)],
                    raw[0:64, :],
                    bc[:],
                    mybir.AluOpType.mult,
                )

        # ---- MLP: h = relu(pair @ W1 + b1); out = sigmoid(h @ W2 + b2) ----
        out_sb = const.tile([1, E_LOC], dt.float32)
        for et in range(N_ET):
            hps = psum.tile([HID, 512], dt.float32, tag="ps")
            rhs_chunks = (
                zsrc[:].bitcast(dt.bfloat16)[0:64, 512 * et:512 * (et + 1)],
                zdst[:].bitcast(dt.bfloat16)[0:64, 512 * et:512 * (et + 1)],
                cn_sb["ddi"][:, 512 * et:512 * (et + 1)],
                cn_sb["dp"][:, 512 * et:512 * (et + 1)],
            )
            for j, rhs in enumerate(rhs_chunks):
                nc.tensor.matmul(
                    hps[:],
                    w1t[:].bitcast(dt.bfloat16)[:, HID * j:HID * (j + 1)],
                    rhs,
                    start=(j == 0),
                    stop=(j == 3),
                )
            hsb = tails.tile([HID, 512], dt.bfloat16, tag="h")
            nc.scalar.activation(
                hsb[:], hps[:], mybir.ActivationFunctionType.Relu, bias=b1t[:, 0:1]
            )
            lps = psum.tile([1, 512], dt.float32, tag="ps")
            nc.tensor.matmul(
                lps[:], w2t[:].bitcast(dt.bfloat16), hsb[:], start=True, stop=True
            )
            nc.scalar.activation(
                out_sb[:, 512 * et:512 * (et + 1)],
                lps[:],
                mybir.ActivationFunctionType.Sigmoid,
                bias=b2t[:, 0:1],
            )
        nc.sync.dma_start(t["OUT"][:, :], out_sb[:])


def build_program():
    nc = bacc.Bacc("TRN2", target_bir_lowering=False)
    dt = mybir.dt
    t = {
        "A1": nc.dram_tensor(
            "A1", [U_PAD, F_DDIZ * 128], dt.uint16, kind="ExternalInput"
        ).ap(),
        "A2": nc.dram_tensor(
            "A2", [U_PAD, F_DP * 128], dt.uint16, kind="ExternalInput"
        ).ap(),
        "IDX": nc.dram_tensor("IDX", [128, IDX_COLS], dt.int16, kind="ExternalInput").ap(),
        "ZD": nc.dram_tensor(
            "ZD", [128, 2 * F_DDI * 2 * MCOL], dt.uint8, kind="ExternalInput"
        ).ap(),
        "ZP": nc.dram_tensor(
            "ZP", [128, 2 * F_DP * 2 * MCOL], dt.uint8, kind="ExternalInput"
        ).ap(),
        "W1": nc.dram_tensor("W1", [64, 4 * HID], dt.uint16, kind="ExternalInput").ap(),
        "B1": nc.dram_tensor("B1", [HID, 1], dt.float32, kind="ExternalInput").ap(),
        "W2": nc.dram_tensor("W2", [HID, 1], dt.uint16, kind="ExternalInput").ap(),
        "B2": nc.dram_tensor("B2", [1, 1], dt.float32, kind="ExternalInput").ap(),
        "OUT": nc.dram_tensor("OUT", [1, E_LOC], dt.float32, kind="ExternalOutput").ap(),
    }
    with tile.TileContext(nc) as tc:
        build_body(tc, t)
    nc.compile()
    return nc


def host_prep(z_drug, z_protein, ddi_ei, dp_ei, pred_ei, W1, b1, W2, b2):
    """Build the 8 per-core input maps (all numpy, no device work)."""
    z_drug = np.asarray(z_drug, np.float32)
    z_protein = np.asarray(z_protein, np.float32)
    ddi_ei = np.asarray(ddi_ei, np.int64)
    dp_ei = np.asarray(dp_ei, np.int64)
    pred_ei = np.asarray(pred_ei, np.int64)

    A_ddi = np.zeros((N_D, N_D), dtype=np.uint8)
    A_ddi[ddi_ei[0], ddi_ei[1]] = 0x38
    A_ddi[ddi_ei[1], ddi_ei[0]] = 0x38
    A_dp = np.zeros((N_D, N_P), dtype=np.uint8)
    A_dp[dp_ei[0], dp_ei[1]] = 0x38

    zb_full = np.zeros((N_D, 128), dtype=np.float32)
    zb_full[:, :D_DIM] = z_drug
    zb_bytes = zb_full.astype(BF16).view(np.uint8)  # [N_D, 256]

    # partition-major layouts for clean single-descriptor-per-partition DMAs
    ZD = np.ascontiguousarray(
        _pack_z_doublerow(z_drug).transpose(1, 0, 2).reshape(128, -1)
    )
    ZP = np.ascontiguousarray(
        _pack_z_doublerow(z_protein).transpose(1, 0, 2).reshape(128, -1)
    )
    W1p = np.ascontiguousarray(
        np.asarray(W1, np.float32)
        .reshape(4, 64, HID)
        .astype(BF16)
        .view(np.uint16)
        .transpose(1, 0, 2)
        .reshape(64, 4 * HID)
    )
    B1 = np.asarray(b1, np.float32).reshape(HID, 1)
    W2p = np.asarray(W2, np.float32).reshape(HID, 1).astype(BF16).view(np.uint16)
    B2 = np.asarray(b2, np.float32).reshape(1, 1)

    in_maps = []
    for c in range(N_CORES):
        s = pred_ei[0, c * E_LOC:(c + 1) * E_LOC]
        d = pred_ei[1, c * E_LOC:(c + 1) * E_LOC]
        rows = np.unique(np.concatenate([s, d]))
        nu = rows.shape[0]
        assert nu <= U_PAD
        remap_s = np.searchsorted(rows, s).astype(np.int16)
        remap_d = np.searchsorted(rows, d).astype(np.int16)
        A1 = np.zeros((U_PAD, F_DDIZ * 256), dtype=np.uint8)
        A1[:nu, : N_D] = A_ddi[rows]
        A1[:nu, N_D:] = zb_bytes[rows]
        A2 = np.zeros((U_PAD, N_P), dtype=np.uint8)
        A2[:nu] = A_dp[rows]

        cols = []
        for g in range(N_G):
            cols.append(_wrap_idxs(remap_s[G_DDI * g:G_DDI * (g + 1)]))
            cols.append(_wrap_idxs(remap_d[G_DDI * g:G_DDI * (g + 1)]))
        for h in range(N_H):
            cols.append(_wrap_idxs(remap_s[G_DP * h:G_DP * (h + 1)]))
            cols.append(_wrap_idxs(remap_d[G_DP * h:G_DP * (h + 1)]))
        idx = np.concatenate(cols, axis=1)
        assert idx.shape == (128, IDX_COLS)

        in_maps.append(
            {
                "A1": A1.view(np.uint16),
                "A2": A2.view(np.uint16),
                "IDX": idx,
                "ZD": ZD,
                "ZP": ZP,
                "W1": W1p,
                "B1": B1,
                "W2": W2p,
                "B2": B2,
            }
        )
    return in_maps


def kernel(z_drug, z_protein, ddi_ei, dp_ei, pred_ei, W1, b1, W2, b2, _profile=None):
    from concourse.bass_utils import run_bass_kernel_spmd

    in_maps = host_prep(z_drug, z_protein, ddi_ei, dp_ei, pred_ei, W1, b1, W2, b2)
    nc = build_program()
    res = run_bass_kernel_spmd(
        nc,
        in_maps,
        core_ids=list(range(N_CORES)),
        **({} if _profile is None else _profile),
    )
    if _profile is not None:
        kernel.last_results = res
    out = np.concatenate([r["OUT"].reshape(-1) for r in res.results])
    return out.astype(np.float32)


# revision 23
# speedup vs baseline: 1.0844x; 1.0844x over previous
"""Trainium2 Bass kernel for EnhancedLinkPredictor (GNN common-neighbor link prediction).

Math (per prediction edge e=(s,d)):
  shared_ddi = adj_ddi[s] & adj_ddi[d]          (drug-drug, N_D=8192)
  cn_ddi     = (shared_ddi @ z_drug)  / max(|shared_ddi|, 1)
  shared_dp  = adj_dp[s]  & adj_dp[d]           (drug-protein, N_P=4096)
  cn_prot    = (shared_dp @ z_protein) / max(|shared_dp|, 1)
  pair  = [z_drug[s], z_drug[d], cn_ddi, cn_prot]   (256)
  out   = sigmoid(relu(pair @ W1 + b1) @ W2 + b2)

Device strategy (8 cores, data-parallel over the 16384 pred edges, 2048/core):
  - Adjacency rows are fp8-coded bytes {0x00, 0x38=1.0fp8}, built host-side and
    compacted per core to the rows that core actually touches (<=4096 unique).
    The ddi row additionally carries the drug's z row as 128 bf16 (256B), so a
    single gather per edge endpoint fetches adjacency + embedding.
  - dma_gather(transpose=True, uint16 view) pulls whole rows per edge endpoint
    and lands them k-pair-per-partition (partition p of 256-k chunk f holds
    k = 256f+2p, 256f+2p+1) - fp8-DoubleRow-ready, no on-chip transpose.
    GPSIMD descriptor generation costs ~8.4ns/idx, so each endpoint is visited
    exactly once per relation (full-row elems, small idx lists per call).
  - AND between s-rows and d-rows is a uint32 bitwise_and on DVE, writing
    128/256-edge column slices of 512-edge masked supertiles.
  - Matmuls run in fp8 DoubleRow with Z split into two e4m3 limbs (hi +
    residual) for ~bf16 accuracy, accumulated in f32 PSUM; Z column 64 is ones
    so the intersection counts fall out of the same matmul.
  - Everything downstream is computed transposed ([dim, edge]) so each
    matmul's contraction dim is on partitions; the MLP consumes pair^T.
"""

import numpy as np
import ml_dtypes
from contextlib import ExitStack

import concourse.bass as bass
import concourse.bacc as bacc
import concourse.mybir as mybir
import concourse.tile as tile

N_D, N_P = 8192, 4096
D_DIM, HID = 64, 128
E_PRED = 16384
N_CORES = 8
E_LOC = E_PRED // N_CORES          # 2048 edges per core
U_PAD = 4096                       # compacted adjacency row count (>= unique refs)

MCOL = 80                          # padded Z columns (64 dims + 1 ones + 15 pad)
KCHUNK = 256                       # k per DoubleRow matmul

F_DDI = N_D // KCHUNK              # 32 adjacency chunks in a ddi row
F_DDIZ = F_DDI + 1                 # + z chunk
F_DP = N_P // KCHUNK               # 16 chunks in a dp row
G_DDI = 128                        # edges per ddi gather call (266 descs)
G_DP = 256                         # edges per dp gather call (258 descs)
N_G = E_LOC // G_DDI               # 16 ddi edge groups
N_H = E_LOC // G_DP                # 8 dp edge groups
N_ET = E_LOC // 512                # 4 supertiles of 512 edges
IDX_COLS = 2 * N_G * (G_DDI // 16) + 2 * N_H * (G_DP // 16)  # 512
USE_DOUBLEROW = True

FP8 = ml_dtypes.float8_e4m3
BF16 = ml_dtypes.bfloat16


def _fp8_limbs(x: np.ndarray):
    hi = x.astype(FP8)
    lo = (x - hi.astype(np.float32)).astype(FP8)
    return hi.view(np.uint8), lo.view(np.uint8)


def _pack_z_doublerow(z: np.ndarray):
    """z [K, 64] f32 -> [2*K/256, 128, 2*MCOL] uint8 fp8-coded DoubleRow lhsT
    groups (limb-major), with a ones column at index 64."""
    K = z.shape[0]
    zext = np.zeros((K, MCOL), dtype=np.float32)
    zext[:, :D_DIM] = z
    zext[:, D_DIM] = 1.0
    hi, lo = _fp8_limbs(zext)
    n_chunks = K // KCHUNK
    out = np.empty((2 * n_chunks, 128, 2 * MCOL), dtype=np.uint8)
    for li, limb in enumerate((hi, lo)):
        out[li * n_chunks:(li + 1) * n_chunks] = limb.reshape(
            n_chunks, 128, 2 * MCOL
        )
    return out


def _wrap_idxs(idx: np.ndarray):
    """[n] int -> [128, n/16] int16 wrapped (j -> [j%16, j//16]) + 8x replicated."""
    n = idx.shape[0]
    w = np.zeros((16, n // 16), dtype=np.int16)
    w[np.arange(n) % 16, np.arange(n) // 16] = idx.astype(np.int16)
    return np.tile(w, (8, 1))


def build_body(tc, t):
    """Emit the per-core program. t: dict name -> AP of DRAM tensors."""
    nc = tc.nc
    dt = mybir.dt
    with ExitStack() as ctx:
        const = ctx.enter_context(tc.tile_pool(name="const", bufs=1))
        gpool = ctx.enter_context(tc.tile_pool(name="gath", bufs=6))
        mpool = ctx.enter_context(tc.tile_pool(name="mask", bufs=2))
        tails = ctx.enter_context(tc.tile_pool(name="tails", bufs=2))
        pairp = ctx.enter_context(tc.tile_pool(name="pair", bufs=1))
        psum = ctx.enter_context(tc.tile_pool(name="ps", bufs=8, space="PSUM"))

        # ---- constants / small loads ----
        # idx on the sync queue (gathers depend on it); bulk constants on the
        # scalar HWDGE queue so they don't delay the first gather.
        idxt = const.tile([128, IDX_COLS], dt.int16)
        nc.sync.dma_start(idxt[:], t["IDX"][:, :])

        zd = const.tile([128, 2 * F_DDI * 2 * MCOL], dt.uint8)
        nc.scalar.dma_start(zd[:], t["ZD"][:, :])
        zp = const.tile([128, 2 * F_DP * 2 * MCOL], dt.uint8)
        nc.scalar.dma_start(zp[:], t["ZP"][:, :])
        w1t = const.tile([64, 4 * HID], dt.uint16)
        nc.scalar.dma_start(w1t[:], t["W1"][:, :])
        w2t = const.tile([128, 1], dt.uint16)
        nc.scalar.dma_start(w2t[:], t["W2"][:, :])
        b1t = const.tile([128, 1], dt.float32)
        nc.scalar.dma_start(b1t[:], t["B1"][:, :])
        b2t = const.tile([1, 1], dt.float32)
        nc.scalar.dma_start(b2t[:], t["B2"][:, :])
        ones = const.tile([1, 64], dt.float32)
        nc.vector.memset(ones[:], 1.0)

        zsrc = pairp.tile([128, E_LOC], dt.uint16)
        zdst = pairp.tile([128, E_LOC], dt.uint16)

        cn_ps = {}
        for et in range(N_ET):
            cn_ps[("ddi", et)] = psum.tile(
                [MCOL, 512], dt.float32, tag="ps", name=f"cn_ddi{et}"
            )
            cn_ps[("dp", et)] = psum.tile(
                [MCOL, 512], dt.float32, tag="ps", name=f"cn_dp{et}"
            )

        def matmuls(rel, et, msk, zt, n_chunks):
            m8 = msk[:].bitcast(dt.float8e4).rearrange(
                "p (f i two) -> p f two i", f=n_chunks, two=2
            )
            zt8 = zt[:].bitcast(dt.float8e4).rearrange(
                "p (g two m) -> p g two m", g=2 * n_chunks, two=2
            )
            for f in range(n_chunks):
                for limb in range(2):
                    g = limb * n_chunks + f
                    first = f == 0 and limb == 0
                    last = f == n_chunks - 1 and limb == 1
                    if USE_DOUBLEROW:
                        nc.tensor.matmul(
                            cn_ps[(rel, et)][:],
                            zt8[:, g, :, :],
                            m8[:, f, :, :],
                            start=first,
                            stop=last,
                            perf_mode=mybir.MatmulPerfMode.DoubleRow,
                        )
                    else:
                        for par in range(2):
                            nc.tensor.matmul(
                                cn_ps[(rel, et)][:],
                                zt8[:, g, par, :],
                                m8[:, f, par, :],
                                start=first and par == 0,
                                stop=last and par == 1,
                            )

        cn_sb = {
            "ddi": pairp.tile([64, E_LOC], dt.bfloat16, tag="cnddi", name="cnddi"),
            "dp": pairp.tile([64, E_LOC], dt.bfloat16, tag="cndp", name="cndp"),
        }
        out_sb = const.tile([1, E_LOC], dt.float32)
        dp_base = 2 * N_G * (G_DDI // 16)

        def ddi_supertile(et):
            msk = mpool.tile([128, F_DDI * 512], dt.uint16, tag="mka", name=f"mka{et}")
            for sub in range(4):
                g = 4 * et + sub
                pair_tiles = []
                for side in range(2):  # 0=s, 1=d
                    gt = gpool.tile([128, F_DDIZ * G_DDI], dt.uint16, tag="gt")
                    col0 = (2 * g + side) * (G_DDI // 16)
                    nc.gpsimd.dma_gather(
                        out_ap=gt[:].rearrange("p (f i) -> p f i", f=F_DDIZ),
                        in_ap=t["A1"][:, :],
                        idxs_ap=idxt[:, col0:col0 + G_DDI // 16],
                        num_idxs=G_DDI,
                        num_idxs_reg=G_DDI,
                        elem_size=F_DDIZ * 128,
                        elem_step=F_DDIZ * 128,
                        transpose=True,
                        single_packet=False,
                    )
                    pair_tiles.append(gt)
                    # z chunk -> pair^T rows
                    ztile = zsrc if side == 0 else zdst
                    nc.vector.tensor_copy(
                        ztile[:, G_DDI * g:G_DDI * (g + 1)],
                        gt[:].rearrange("p (f i) -> p f i", f=F_DDIZ)[:, F_DDI, :],
                    )
                s32 = pair_tiles[0][:].bitcast(dt.uint32).rearrange(
                    "p (f i) -> p f i", f=F_DDIZ
                )
                d32 = pair_tiles[1][:].bitcast(dt.uint32).rearrange(
                    "p (f i) -> p f i", f=F_DDIZ
                )
                m32 = msk[:].bitcast(dt.uint32).rearrange("p (f i) -> p f i", f=F_DDI)
                nw = G_DDI // 2
                nc.vector.tensor_tensor(
                    m32[:, :, sub * nw:(sub + 1) * nw],
                    s32[:, :F_DDI, :],
                    d32[:, :F_DDI, :],
                    mybir.AluOpType.bitwise_and,
                )
            matmuls("ddi", et, msk, zd, F_DDI)

        def dp_supertile(et):
            msk = mpool.tile([128, F_DP * 512], dt.uint16, tag="mkb", name=f"mkb{et}")
            for sub in range(2):
                h = 2 * et + sub
                pair_tiles = []
                for side in range(2):
                    gt = gpool.tile([128, F_DP * G_DP], dt.uint16, tag="gt")
                    col0 = dp_base + (2 * h + side) * (G_DP // 16)
                    nc.gpsimd.dma_gather(
                        out_ap=gt[:].rearrange("p (f i) -> p f i", f=F_DP),
                        in_ap=t["A2"][:, :],
                        idxs_ap=idxt[:, col0:col0 + G_DP // 16],
                        num_idxs=G_DP,
                        num_idxs_reg=G_DP,
                        elem_size=F_DP * 128,
                        elem_step=F_DP * 128,
                        transpose=True,
                        single_packet=False,
                    )
                    pair_tiles.append(gt)
                s32 = pair_tiles[0][:].bitcast(dt.uint32).rearrange(
                    "p (f i) -> p f i", f=F_DP
                )
                d32 = pair_tiles[1][:].bitcast(dt.uint32).rearrange(
                    "p (f i) -> p f i", f=F_DP
                )
                m32 = msk[:].bitcast(dt.uint32).rearrange("p (f i) -> p f i", f=F_DP)
                nw = G_DP // 2
                nc.vector.tensor_tensor(
                    m32[:, :, sub * nw:(sub + 1) * nw],
                    s32[:, :, :],
                    d32[:, :, :],
                    mybir.AluOpType.bitwise_and,
                )
            matmuls("dp", et, msk, zp, F_DP)

        def normalize(rel, et):
            ps = cn_ps[(rel, et)]
            raw = tails.tile([66, 512], dt.float32, tag="raw")
            nc.scalar.copy(raw[:], ps[0:66, :])
            rec = tails.tile([1, 512], dt.float32, tag="rec")
            nc.vector.tensor_scalar_max(rec[:], raw[64:65, :], 1.0)
            nc.vector.reciprocal(rec[:], rec[:])
            bc = psum.tile([64, 512], dt.float32, tag="ps")
            nc.tensor.matmul(bc[:], ones[:], rec[:], start=True, stop=True)
            nc.vector.tensor_tensor(
                cn_sb[rel][:, 512 * et:512 * (et + 1)],
                raw[0:64, :],
                bc[:],
                mybir.AluOpType.mult,
            )

        def mlp(et):
            hps = psum.tile([HID, 512], dt.float32, tag="ps")
            rhs_chunks = (
                zsrc[:].bitcast(dt.bfloat16)[0:64, 512 * et:512 * (et + 1)],
                zdst[:].bitcast(dt.bfloat16)[0:64, 512 * et:512 * (et + 1)],
                cn_sb["ddi"][:, 512 * et:512 * (et + 1)],
                cn_sb["dp"][:, 512 * et:512 * (et + 1)],
            )
            for j, rhs in enumerate(rhs_chunks):
                nc.tensor.matmul(
                    hps[:],
                    w1t[:].bitcast(dt.bfloat16)[:, HID * j:HID * (j + 1)],
                    rhs,
                    start=(j == 0),
                    stop=(j == 3),
                )
            hsb = tails.tile([HID, 512], dt.bfloat16, tag="h")
            nc.scalar.activation(
                hsb[:], hps[:], mybir.ActivationFunctionType.Relu, bias=b1t[:, 0:1]
            )
            lps = psum.tile([1, 512], dt.float32, tag="ps")
            nc.tensor.matmul(
                lps[:], w2t[:].bitcast(dt.bfloat16), hsb[:], start=True, stop=True
            )
            nc.scalar.activation(
                out_sb[:, 512 * et:512 * (et + 1)],
                lps[:],
                mybir.ActivationFunctionType.Sigmoid,
                bias=b2t[:, 0:1],
            )

        # interleave: per supertile do ddi + dp, then normalize + MLP as soon
        # as that supertile's accumulations are complete
        for et in range(N_ET):
            ddi_supertile(et)
            dp_supertile(et)
            normalize("ddi", et)
            normalize("dp", et)
            mlp(et)
        nc.sync.dma_start(t["OUT"][:, :], out_sb[:])


def build_program():
    nc = bacc.Bacc("TRN2", target_bir_lowering=False)
    dt = mybir.dt
    t = {
        "A1": nc.dram_tensor(
            "A1", [U_PAD, F_DDIZ * 128], dt.uint16, kind="ExternalInput"
        ).ap(),
        "A2": nc.dram_tensor(
            "A2", [U_PAD, F_DP * 128], dt.uint16, kind="ExternalInput"
        ).ap(),
        "IDX": nc.dram_tensor("IDX", [128, IDX_COLS], dt.int16, kind="ExternalInput").ap(),
        "ZD": nc.dram_tensor(
            "ZD", [128, 2 * F_DDI * 2 * MCOL], dt.uint8, kind="ExternalInput"
        ).ap(),
        "ZP": nc.dram_tensor(
            "ZP", [128, 2 * F_DP * 2 * MCOL], dt.uint8, kind="ExternalInput"
        ).ap(),
        "W1": nc.dram_tensor("W1", [64, 4 * HID], dt.uint16, kind="ExternalInput").ap(),
        "B1": nc.dram_tensor("B1", [HID, 1], dt.float32, kind="ExternalInput").ap(),
        "W2": nc.dram_tensor("W2", [HID, 1], dt.uint16, kind="ExternalInput").ap(),
        "B2": nc.dram_tensor("B2", [1, 1], dt.float32, kind="ExternalInput").ap(),
        "OUT": nc.dram_tensor("OUT", [1, E_LOC], dt.float32, kind="ExternalOutput").ap(),
    }
    with tile.TileContext(nc) as tc:
        build_body(tc, t)
    nc.compile()
    return nc


def host_prep(z_drug, z_protein, ddi_ei, dp_ei, pred_ei, W1, b1, W2, b2):
    """Build the 8 per-core input maps (all numpy, no device work)."""
    z_drug = np.asarray(z_drug, np.float32)
    z_protein = np.asarray(z_protein, np.float32)
    ddi_ei = np.asarray(ddi_ei, np.int64)
    dp_ei = np.asarray(dp_ei, np.int64)
    pred_ei = np.asarray(pred_ei, np.int64)

    A_ddi = np.zeros((N_D, N_D), dtype=np.uint8)
    A_ddi[ddi_ei[0], ddi_ei[1]] = 0x38
    A_ddi[ddi_ei[1], ddi_ei[0]] = 0x38
    A_dp = np.zeros((N_D, N_P), dtype=np.uint8)
    A_dp[dp_ei[0], dp_ei[1]] = 0x38

    zb_full = np.zeros((N_D, 128), dtype=np.float32)
    zb_full[:, :D_DIM] = z_drug
    zb_bytes = zb_full.astype(BF16).view(np.uint8)  # [N_D, 256]

    # partition-major layouts for clean single-descriptor-per-partition DMAs
    ZD = np.ascontiguousarray(
        _pack_z_doublerow(z_drug).transpose(1, 0, 2).reshape(128, -1)
    )
    ZP = np.ascontiguousarray(
        _pack_z_doublerow(z_protein).transpose(1, 0, 2).reshape(128, -1)
    )
    W1p = np.ascontiguousarray(
        np.asarray(W1, np.float32)
        .reshape(4, 64, HID)
        .astype(BF16)
        .view(np.uint16)
        .transpose(1, 0, 2)
        .reshape(64, 4 * HID)
    )
    B1 = np.asarray(b1, np.float32).reshape(HID, 1)
    W2p = np.asarray(W2, np.float32).reshape(HID, 1).astype(BF16).view(np.uint16)
    B2 = np.asarray(b2, np.float32).reshape(1, 1)

    in_maps = []
    for c in range(N_CORES):
        s = pred_ei[0, c * E_LOC:(c + 1) * E_LOC]
        d = pred_ei[1, c * E_LOC:(c + 1) * E_LOC]
        rows = np.unique(np.concatenate([s, d]))
        nu = rows.shape[0]
        assert nu <= U_PAD
        remap_s = np.searchsorted(rows, s).astype(np.int16)
        remap_d = np.searchsorted(rows, d).astype(np.int16)
        A1 = np.zeros((U_PAD, F_DDIZ * 256), dtype=np.uint8)
        A1[:nu, : N_D] = A_ddi[rows]
        A1[:nu, N_D:] = zb_bytes[rows]
        A2 = np.zeros((U_PAD, N_P), dtype=np.uint8)
        A2[:nu] = A_dp[rows]

        cols = []
        for g in range(N_G):
            cols.append(_wrap_idxs(remap_s[G_DDI * g:G_DDI * (g + 1)]))
            cols.append(_wrap_idxs(remap_d[G_DDI * g:G_DDI * (g + 1)]))
        for h in range(N_H):
            cols.append(_wrap_idxs(remap_s[G_DP * h:G_DP * (h + 1)]))
            cols.append(_wrap_idxs(remap_d[G_DP * h:G_DP * (h + 1)]))
        idx = np.concatenate(cols, axis=1)
        assert idx.shape == (128, IDX_COLS)

        in_maps.append(
            {
                "A1": A1.view(np.uint16),
                "A2": A2.view(np.uint16),
                "IDX": idx,
                "ZD": ZD,
                "ZP": ZP,
                "W1": W1p,
                "B1": B1,
                "W2": W2p,
                "B2": B2,
            }
        )
    return in_maps


def kernel(z_drug, z_protein, ddi_ei, dp_ei, pred_ei, W1, b1, W2, b2, _profile=None):
    from concourse.bass_utils import run_bass_kernel_spmd

    in_maps = host_prep(z_drug, z_protein, ddi_ei, dp_ei, pred_ei, W1, b1, W2, b2)
    nc = build_program()
    res = run_bass_kernel_spmd(
        nc,
        in_maps,
        core_ids=list(range(N_CORES)),
        **({} if _profile is None else _profile),
    )
    if _profile is not None:
        kernel.last_results = res
    out = np.concatenate([r["OUT"].reshape(-1) for r in res.results])
    return out.astype(np.float32)
